# revision 10
# baseline (speedup 1.0000x reference)
"""CRF NLL loss kernel for 8 Trainium2 NeuronCores — time-sharded forward algorithm.

Math: exp-domain forward recurrence alpha_{s+1} = diag(em_s) M alpha_s with
M = exp(transitions), em prescaled per step by its LSE so fp32/bf16 never
over/underflows. logZ(b) = log(w . alpha_{L_b}) with w = exp(trans[STOP]).

Sharding: TIME-sharded (not batch): each core owns a 128-step range of ALL 512
sequences, split into C=6 chains of 31 steps. A chain's stream starts ~9-11
steps before its owned block; the CRF transfer recurrence contracts initial-
condition error by ~0.2x/step (measured), so after the warmup prefix the state
direction is exact to ~1e-7 and only an unknown per-sequence log-scale remains.
The host stitches those scales chain-to-chain through overlap records. Chain 0
of core 0 starts from the exact one-hot START state (no warmup).

Layout: two 256-sequence groups packed on partitions 0-47 / 48-95 plus two
stop-dot rows (96/97) via a block-diagonal [96,98] weight augmented with the
STOP row. Chains are PAIRED: a pair shares one PSUM bank [98,512] (each
chain-half writes 256 columns) so one [98,512] elementwise multiply retires
two chains' steps, halving per-instruction PSUM access penalties and
instruction count. Per pair-step: 2 PE matmuls (bf16, fp32 psum) + one
PSUM->SBUF multiply on a rotating engine path (fused DVE / Act-copy + 2x DVE
mul / Act-copy + GPSIMD mul) to balance engine busy time. Records (rows 96/97
of every ring slot) are DMA'd out; the host converts them to logZ and
subtracts the gold path score.
"""
import os
import sys

import numpy as np

for _p in ("/opt/trn_rl_repo", "/root/.axon_site/_ro/trn_rl_repo"):
    if os.path.isdir(_p) and _p not in sys.path:
        sys.path.insert(0, _p)

import ml_dtypes

import concourse.bacc as bacc
import concourse.tile as tile
from concourse import mybir
from concourse import bass_utils

BF16NP = ml_dtypes.bfloat16

B, S, T = 512, 1024, 48
START, STOP, PAD = 45, 46, 47
NCORE = 8
C = 6                    # chains (time blocks) per core
NPAIR = 3
NST = 31                 # steps per chain
NSLOT = NST + 1          # emis slots per chain (init + 31 em steps)
F = 256                  # sequences per partition-group (one chain-half)
FP = 512                 # pair free width (two chain-halves)
P = 98                   # partitions: 48 tags x 2 groups + 2 stop rows
F32 = mybir.dt.float32
BF16 = mybir.dt.bfloat16

# multiply-path schedule per (pair, round): D = fused DVE (psum x em),
# A = Act copy + 2x-mode DVE mul, P = Act copy + GPSIMD mul.
PAT = "DADPDADDAPD"

# emis chunking: chunk 0 = slots [0,2) (small, cuts start stagger), chunks
# 1..5 = 6 slots each, double-buffered (odd chunk -> cols [6:12) slots).
def _chunk_of_slot(s):
    return 0 if s < 2 else (s - 2) // 6 + 1


def _col_of_slot(s):
    if s < 2:
        return s * FP
    k = (s - 2) // 6 + 1
    off = (s - 2) % 6
    return ((k % 2) * 6 + off) * FP


def _chunk_slots(k):
    return (0, 2) if k == 0 else (2 + 6 * (k - 1), 2 + 6 * k)

_CACHE = {}


def _build_program():
    nc = bacc.Bacc(
        "TRN2",
        target_bir_lowering=False,
        debug=False,
        enable_asserts=False,
        num_devices=NCORE,
    )
    emis_d = nc.dram_tensor(
        "emis", [NPAIR, P, NSLOT * FP], BF16, kind="ExternalInput"
    ).ap()
    w_d = nc.dram_tensor("wts", [96, P], BF16, kind="ExternalInput").ap()
    rec_d = nc.dram_tensor("recs", [NPAIR, 2, NST * FP], BF16, kind="ExternalOutput").ap()

    with tile.TileContext(nc) as tc:
        with tc.tile_pool(name="main", bufs=1) as pool, tc.tile_pool(
            name="ps", bufs=1, space="PSUM"
        ) as pp:
            wt = pool.tile([96, P], BF16)
            nc.sync.dma_start(out=wt[:, :], in_=w_d[:, :])
            rings = [
                pool.tile([P, NST * FP], BF16, tag=f"ring{p}", name=f"ring{p}")
                for p in range(NPAIR)
            ]
            ems = [
                pool.tile([P, 12 * FP], BF16, tag=f"em{p}", name=f"em{p}")
                for p in range(NPAIR)
            ]
            scr = [
                pool.tile([P, 2 * FP], BF16, tag=f"scr{p}", name=f"scr{p}")
                for p in range(NPAIR)
            ]

            def issue_chunk(p, k):
                s0, s1 = _chunk_slots(k)
                c0 = _col_of_slot(s0)
                nc.sync.dma_start(
                    out=ems[p][:, c0 : c0 + (s1 - s0) * FP],
                    in_=emis_d[p, :, s0 * FP : s1 * FP],
                )

            for k in (0, 1):
                for p in range(NPAIR):
                    issue_chunk(p, k)

            # chunk k overwrites the buffer holding chunk k-2, so it may only
            # be ISSUED after the program-order point where chunk k-2's last
            # reader was emitted (the tile framework resolves reads against
            # the latest prior writer).
            chunk_issue_round = {1: 2, 7: 3, 13: 4, 19: 5}
            for i in range(NST):
                for p in range(NPAIR):
                    if i in chunk_issue_round:
                        issue_chunk(p, chunk_issue_round[i])
                    ps = pp.tile([P, FP], F32, tag=f"mm{p}")
                    for h in (0, 1):
                        if i == 0:
                            src = ems[p][0:96, h * F : h * F + F]
                        else:
                            src = rings[p][
                                0:96, (i - 1) * FP + h * F : (i - 1) * FP + h * F + F
                            ]
                        nc.tensor.matmul(
                            ps[:, h * F : h * F + F],
                            wt[:, :],
                            src,
                            start=True,
                            stop=True,
                        )
                    dst = rings[p][:, i * FP : (i + 1) * FP]
                    cs = _col_of_slot(i + 1)
                    emsl = ems[p][:, cs : cs + FP]
                    mv = PAT[(i + 4 * p) % len(PAT)]
                    if mv == "D":
                        nc.vector.tensor_mul(dst, ps[:, :], emsl)
                    else:
                        sc = scr[p][:, (i % 2) * FP : (i % 2) * FP + FP]
                        nc.scalar.copy(sc, ps[:, :])
                        if mv == "P":
                            nc.gpsimd.tensor_mul(dst, sc, emsl)
                        else:
                            nc.vector.tensor_mul(dst, sc, emsl)
                    if i == 15:
                        nc.sync.dma_start(
                            out=rec_d[p, :, 0 : 16 * FP],
                            in_=rings[p][96:98, 0 : 16 * FP],
                        )
                    elif i == NST - 1:
                        nc.sync.dma_start(
                            out=rec_d[p, :, 16 * FP : NST * FP],
                            in_=rings[p][96:98, 16 * FP : NST * FP],
                        )

    nc.compile()
    return nc


def _blocks_for_core(k):
    """(a, t0, t1) per chain: stream = em steps [a, a+31); owned = (t0, t1]."""
    owned = [30, 20, 20, 20, 19, 19] if k == 0 else [22, 22, 21, 21, 21, 21]
    out = []
    t1 = 128 * k
    for o in owned:
        t1 += o
        out.append((t1 - 30, t1 - o, t1))
    return out


def kernel(feats, masks, tags, transitions):
    feats = np.asarray(feats, dtype=np.float32)
    masks = np.asarray(masks, dtype=np.float32)
    tags = np.asarray(tags)
    trans = np.asarray(transitions, dtype=np.float32)

    if "nc" not in _CACHE:
        _CACHE["nc"] = _build_program()
    nc = _CACHE["nc"]

    lengths = masks.sum(1).astype(np.int64)

    # host prescale: em = exp(feats - LSE_tags(feats)); cumulative C added back
    mx = feats.max(2)
    Kp = np.log(np.exp(feats - mx[:, :, None]).sum(2)) + mx
    Cc = np.zeros((B, S + 1), np.float64)
    Cc[:, 1:] = np.cumsum(Kp.astype(np.float64), 1)
    em = np.exp(feats - Kp[:, :, None].astype(np.float32))

    # packed per-step emission pages [S+1, 98, 256] (page S is a dummy for the
    # one-past-the-end step of the final chain)
    base = np.ones((S + 1, P, F), np.float32)
    base[:S, 0:48] = em[0:F].transpose(1, 2, 0)
    base[:S, 48:96] = em[F:B].transpose(1, 2, 0)

    Mexp = np.exp(trans.astype(np.float64))
    w = np.exp(trans[STOP].astype(np.float64))
    W2 = np.zeros((96, P), np.float64)
    W2[0:48, 0:48] = Mexp.T
    W2[48:96, 48:96] = Mexp.T
    W2[0:48, 96] = w
    W2[48:96, 97] = w
    wts = W2.astype(BF16NP)

    init_uni = np.zeros((P, F), np.float32)
    init_uni[0:96] = 1.0
    init_exact = np.zeros((P, F), np.float32)
    init_exact[START] = 1.0
    init_exact[48 + START] = 1.0

    in_maps = []
    for k in range(NCORE):
        blocks = _blocks_for_core(k)
        # emis[p] slot layout: [NSLOT slots x 512] where cols h*256..h*256+255
        # of slot s belong to chain 2p+h (slot 0 = init, slots 1.. = em pages)
        emv = np.empty((NPAIR, P, NSLOT, 2, F), dtype=BF16NP)
        for c, (a, t0, t1) in enumerate(blocks):
            p, h = divmod(c, 2)
            ini = init_exact if (k == 0 and a == 0) else init_uni
            emv[p, :, 0, h] = ini.astype(BF16NP)
            sl = base[a : a + NST].transpose(1, 0, 2)  # [98, 31, 256]
            emv[p, :, 1:, h] = sl.astype(BF16NP)
        in_maps.append({"emis": emv.reshape(NPAIR, P, NSLOT * FP), "wts": wts})

    _CACHE["in_maps"] = in_maps
    res = bass_utils.run_bass_kernel_spmd(nc, in_maps, core_ids=list(range(NCORE)))
    results = res.results

    # host: stitch per-chain scale offsets, read logZ at L, subtract gold
    chains = []
    for k in range(NCORE):
        rec = (
            np.asarray(results[k]["recs"])
            .astype(np.float64)
            .reshape(NPAIR, 2, NST, 2, F)
        )
        for c, (a, t0, t1) in enumerate(_blocks_for_core(k)):
            p, h = divmod(c, 2)
            chains.append((a, t0, t1, rec[p, :, :, h, :]))
    chains.sort(key=lambda x: x[2])

    grp = np.arange(B) // F
    lane = np.arange(B) % F

    def logr(rc, t, a):
        return np.log(np.maximum(rc[grp, t - a, lane], 1e-300))

    g_off = np.zeros(B)
    logZ = np.full(B, np.nan)
    prev = None
    for (a, t0, t1, rc) in chains:
        if prev is not None:
            pa, _, _, prc = prev
            lt_prev = logr(prc, t0, pa) + Cc[:, t0] - Cc[:, pa] + g_off
            g_off = lt_prev - (logr(rc, t0, a) + Cc[:, t0] - Cc[:, a])
        sel = (lengths > t0) & (lengths <= t1)
        if sel.any():
            Ls = lengths[sel]
            logZ[sel] = (
                np.log(np.maximum(rc[grp[sel], Ls - a, lane[sel]], 1e-300))
                + Cc[sel, Ls]
                - Cc[sel, a]
                + g_off[sel]
            )
        prev = (a, t0, t1, rc)

    bi = np.arange(B)
    em_g = feats[bi[:, None], np.arange(S)[None, :], tags].astype(np.float64)
    tags_ext = np.concatenate([np.full((B, 1), START, tags.dtype), tags], 1)
    trsc = trans.astype(np.float64)[tags_ext[:, 1:], tags_ext[:, :-1]]
    gold = ((em_g + trsc) * masks.astype(np.float64)).sum(1) + trans[
        STOP, tags_ext[bi, lengths]
    ].astype(np.float64)
    return (logZ - gold).astype(np.float32)


# revision 11
# speedup vs baseline: 1.1088x; 1.1088x over previous
"""CRF NLL loss kernel for 8 Trainium2 NeuronCores — time-sharded forward algorithm.

Math: exp-domain forward recurrence alpha_{s+1} = diag(em_s) M alpha_s with
M = exp(transitions), em prescaled per step by its LSE so fp32/bf16 never
over/underflows. logZ(b) = log(w . alpha_{L_b}) with w = exp(trans[STOP]).

Sharding: TIME-sharded (not batch): each core owns a 128-step range of ALL 512
sequences, split into C=6 chains of 31 steps. A chain's stream starts ~9-11
steps before its owned block; the CRF transfer recurrence contracts initial-
condition error by ~0.2x/step (measured), so after the warmup prefix the state
direction is exact to ~1e-7 and only an unknown per-sequence log-scale remains.
The host stitches those scales chain-to-chain through overlap records. Chain 0
of core 0 starts from the exact one-hot START state (no warmup).

Layout: two 256-sequence groups packed on partitions 0-47 / 48-95 plus two
stop-dot rows (96/97) via a block-diagonal [96,98] weight augmented with the
STOP row. Chains are PAIRED: a pair shares one PSUM bank [98,512] (each
chain-half writes 256 columns) so one [98,512] elementwise multiply retires
two chains' steps, halving per-instruction PSUM access penalties and
instruction count. Per pair-step: 2 PE matmuls (bf16, fp32 psum) + one
PSUM->SBUF multiply on a rotating engine path (fused DVE / Act-copy + 2x DVE
mul / Act-copy + GPSIMD mul) to balance engine busy time. Records (rows 96/97
of every ring slot) are DMA'd out; the host converts them to logZ and
subtracts the gold path score.
"""
import os
import sys

import numpy as np

for _p in ("/opt/trn_rl_repo", "/root/.axon_site/_ro/trn_rl_repo"):
    if os.path.isdir(_p) and _p not in sys.path:
        sys.path.insert(0, _p)

import ml_dtypes

import concourse.bacc as bacc
import concourse.tile as tile
from concourse import mybir
from concourse import bass_utils

BF16NP = ml_dtypes.bfloat16

B, S, T = 512, 1024, 48
START, STOP, PAD = 45, 46, 47
NCORE = 8
C = 6                    # chains (time blocks) per core
NPAIR = 3
NST = 30                 # steps per chain
NSLOT = NST + 1          # emis slots per chain (init + 31 em steps)
F = 256                  # sequences per partition-group (one chain-half)
FP = 512                 # pair free width (two chain-halves)
P = 98                   # partitions: 48 tags x 2 groups + 2 stop rows
F32 = mybir.dt.float32
BF16 = mybir.dt.bfloat16

# multiply-path schedule per (pair, round): D = fused DVE (psum x em),
# A = Act copy + 2x-mode DVE mul, P = Act copy + GPSIMD mul.
PAT = "DADPDADDAPD"

# emis chunking: chunk 0 = slots [0,2) (small, cuts start stagger), chunks
# 1..5 = 6 slots each (last one 5), triple-buffered: buffer b = k %% 3 at cols
# [6b, 6b+6) slots; chunk 0 nests inside buffer 0.
def _col_of_slot(s):
    if s < 2:
        return s * FP
    k = (s - 2) // 6 + 1
    off = (s - 2) % 6
    return ((k % 3) * 6 + off) * FP


def _chunk_slots(k):
    return (0, 2) if k == 0 else (2 + 6 * (k - 1), min(2 + 6 * k, NSLOT))

_CACHE = {}


def _build_program():
    nc = bacc.Bacc(
        "TRN2",
        target_bir_lowering=False,
        debug=False,
        enable_asserts=False,
        num_devices=NCORE,
    )
    emis_d = nc.dram_tensor(
        "emis", [NPAIR, P, NSLOT * FP], BF16, kind="ExternalInput"
    ).ap()
    w_d = nc.dram_tensor("wts", [96, P], BF16, kind="ExternalInput").ap()
    rec_d = nc.dram_tensor("recs", [NPAIR, 2, NST * FP], BF16, kind="ExternalOutput").ap()

    with tile.TileContext(nc) as tc:
        with tc.tile_pool(name="main", bufs=1) as pool, tc.tile_pool(
            name="ps", bufs=1, space="PSUM"
        ) as pp:
            wt = pool.tile([96, P], BF16)
            nc.sync.dma_start(out=wt[:, :], in_=w_d[:, :])
            rings = [
                pool.tile([P, NST * FP], BF16, tag=f"ring{p}", name=f"ring{p}")
                for p in range(NPAIR)
            ]
            ems = [
                pool.tile([P, 18 * FP], BF16, tag=f"em{p}", name=f"em{p}")
                for p in range(NPAIR)
            ]
            scr = [
                pool.tile([P, 2 * FP], BF16, tag=f"scr{p}", name=f"scr{p}")
                for p in range(NPAIR)
            ]

            def issue_chunk(p, k):
                s0, s1 = _chunk_slots(k)
                c0 = _col_of_slot(s0)
                nc.sync.dma_start(
                    out=ems[p][:, c0 : c0 + (s1 - s0) * FP],
                    in_=emis_d[p, :, s0 * FP : s1 * FP],
                )

            for k in (0, 1, 2):
                for p in range(NPAIR):
                    issue_chunk(p, k)

            # chunk k overwrites the buffer holding chunk k-2, so it may only
            # be ISSUED after the program-order point where chunk k-2's last
            # reader was emitted (the tile framework resolves reads against
            # the latest prior writer).
            chunk_issue_round = {2: 3, 7: 4, 13: 5}
            for i in range(NST):
                for p in range(NPAIR):
                    if i in chunk_issue_round:
                        issue_chunk(p, chunk_issue_round[i])
                    ps = pp.tile([P, FP], F32, tag=f"mm{p}")
                    if i == 0:
                        mm_src = ems[p][0:96, 0:FP]
                    else:
                        mm_src = rings[p][0:96, (i - 1) * FP : i * FP]
                    nc.tensor.matmul(ps[:, :], wt[:, :], mm_src, start=True, stop=True)
                    dst = rings[p][:, i * FP : (i + 1) * FP]
                    cs = _col_of_slot(i + 1)
                    emsl = ems[p][:, cs : cs + FP]
                    mv = PAT[(i + 4 * p) % len(PAT)]
                    if mv == "D":
                        nc.vector.tensor_mul(dst, ps[:, :], emsl)
                    else:
                        sc = scr[p][:, (i % 2) * FP : (i % 2) * FP + FP]
                        nc.scalar.copy(sc, ps[:, :])
                        if mv == "P":
                            nc.gpsimd.tensor_mul(dst, sc, emsl)
                        else:
                            nc.vector.tensor_mul(dst, sc, emsl)
                    if i == 15:
                        nc.sync.dma_start(
                            out=rec_d[p, :, 0 : 16 * FP],
                            in_=rings[p][96:98, 0 : 16 * FP],
                        )
                    elif i == NST - 1:
                        nc.sync.dma_start(
                            out=rec_d[p, :, 16 * FP : NST * FP],
                            in_=rings[p][96:98, 16 * FP : NST * FP],
                        )

    nc.compile()
    return nc


def _blocks_for_core(k):
    """(a, t0, t1) per chain: stream = em steps [a, a+NST); owned = (t0, t1]."""
    owned = [29, 20, 20, 20, 20, 19] if k == 0 else [22, 22, 21, 21, 21, 21]
    out = []
    t1 = 128 * k
    for o in owned:
        t1 += o
        out.append((t1 - (NST - 1), t1 - o, t1))
    return out


def kernel(feats, masks, tags, transitions):
    feats = np.asarray(feats, dtype=np.float32)
    masks = np.asarray(masks, dtype=np.float32)
    tags = np.asarray(tags)
    trans = np.asarray(transitions, dtype=np.float32)

    if "nc" not in _CACHE:
        _CACHE["nc"] = _build_program()
    nc = _CACHE["nc"]

    lengths = masks.sum(1).astype(np.int64)

    # host prescale: em = exp(feats - LSE_tags(feats)); cumulative C added back
    mx = feats.max(2)
    Kp = np.log(np.exp(feats - mx[:, :, None]).sum(2)) + mx
    Cc = np.zeros((B, S + 1), np.float64)
    Cc[:, 1:] = np.cumsum(Kp.astype(np.float64), 1)
    em = np.exp(feats - Kp[:, :, None].astype(np.float32))

    # packed per-step emission pages [S+1, 98, 256] (page S is a dummy for the
    # one-past-the-end step of the final chain)
    base = np.ones((S + 1, P, F), np.float32)
    base[:S, 0:48] = em[0:F].transpose(1, 2, 0)
    base[:S, 48:96] = em[F:B].transpose(1, 2, 0)

    Mexp = np.exp(trans.astype(np.float64))
    w = np.exp(trans[STOP].astype(np.float64))
    W2 = np.zeros((96, P), np.float64)
    W2[0:48, 0:48] = Mexp.T
    W2[48:96, 48:96] = Mexp.T
    W2[0:48, 96] = w
    W2[48:96, 97] = w
    wts = W2.astype(BF16NP)

    init_uni = np.zeros((P, F), np.float32)
    init_uni[0:96] = 1.0
    init_exact = np.zeros((P, F), np.float32)
    init_exact[START] = 1.0
    init_exact[48 + START] = 1.0

    in_maps = []
    for k in range(NCORE):
        blocks = _blocks_for_core(k)
        # emis[p] slot layout: [NSLOT slots x 512] where cols h*256..h*256+255
        # of slot s belong to chain 2p+h (slot 0 = init, slots 1.. = em pages)
        emv = np.empty((NPAIR, P, NSLOT, 2, F), dtype=BF16NP)
        for c, (a, t0, t1) in enumerate(blocks):
            p, h = divmod(c, 2)
            ini = init_exact if (k == 0 and a == 0) else init_uni
            emv[p, :, 0, h] = ini.astype(BF16NP)
            sl = base[a : a + NST].transpose(1, 0, 2)  # [98, 31, 256]
            emv[p, :, 1:, h] = sl.astype(BF16NP)
        in_maps.append({"emis": emv.reshape(NPAIR, P, NSLOT * FP), "wts": wts})

    _CACHE["in_maps"] = in_maps
    res = bass_utils.run_bass_kernel_spmd(nc, in_maps, core_ids=list(range(NCORE)))
    results = res.results

    # host: stitch per-chain scale offsets, read logZ at L, subtract gold
    chains = []
    for k in range(NCORE):
        rec = (
            np.asarray(results[k]["recs"])
            .astype(np.float64)
            .reshape(NPAIR, 2, NST, 2, F)
        )
        for c, (a, t0, t1) in enumerate(_blocks_for_core(k)):
            p, h = divmod(c, 2)
            chains.append((a, t0, t1, rec[p, :, :, h, :]))
    chains.sort(key=lambda x: x[2])

    grp = np.arange(B) // F
    lane = np.arange(B) % F

    def logr(rc, t, a):
        return np.log(np.maximum(rc[grp, t - a, lane], 1e-300))

    g_off = np.zeros(B)
    logZ = np.full(B, np.nan)
    prev = None
    for (a, t0, t1, rc) in chains:
        if prev is not None:
            pa, _, _, prc = prev
            lt_prev = logr(prc, t0, pa) + Cc[:, t0] - Cc[:, pa] + g_off
            g_off = lt_prev - (logr(rc, t0, a) + Cc[:, t0] - Cc[:, a])
        sel = (lengths > t0) & (lengths <= t1)
        if sel.any():
            Ls = lengths[sel]
            logZ[sel] = (
                np.log(np.maximum(rc[grp[sel], Ls - a, lane[sel]], 1e-300))
                + Cc[sel, Ls]
                - Cc[sel, a]
                + g_off[sel]
            )
        prev = (a, t0, t1, rc)

    bi = np.arange(B)
    em_g = feats[bi[:, None], np.arange(S)[None, :], tags].astype(np.float64)
    tags_ext = np.concatenate([np.full((B, 1), START, tags.dtype), tags], 1)
    trsc = trans.astype(np.float64)[tags_ext[:, 1:], tags_ext[:, :-1]]
    gold = ((em_g + trsc) * masks.astype(np.float64)).sum(1) + trans[
        STOP, tags_ext[bi, lengths]
    ].astype(np.float64)
    return (logZ - gold).astype(np.float32)


# revision 12
# speedup vs baseline: 1.2448x; 1.1226x over previous
"""CRF NLL loss kernel for 8 Trainium2 NeuronCores — time-sharded forward algorithm.

Math: exp-domain forward recurrence alpha_{s+1} = diag(em_s) M alpha_s with
M = exp(transitions), em prescaled per step by its LSE so fp32/bf16 never
over/underflows. logZ(b) = log(w . alpha_{L_b}) with w = exp(trans[STOP]).

Sharding: TIME-sharded (not batch): each core owns a 128-step range of ALL 512
sequences, split into C=6 chains of 31 steps. A chain's stream starts ~9-11
steps before its owned block; the CRF transfer recurrence contracts initial-
condition error by ~0.2x/step (measured), so after the warmup prefix the state
direction is exact to ~1e-7 and only an unknown per-sequence log-scale remains.
The host stitches those scales chain-to-chain through overlap records. Chain 0
of core 0 starts from the exact one-hot START state (no warmup).

Layout: two 256-sequence groups packed on partitions 0-47 / 48-95 plus two
stop-dot rows (96/97) via a block-diagonal [96,98] weight augmented with the
STOP row. Chains are PAIRED: a pair shares one PSUM bank [98,512] (each
chain-half writes 256 columns) so one [98,512] elementwise multiply retires
two chains' steps, halving per-instruction PSUM access penalties and
instruction count. Per pair-step: 2 PE matmuls (bf16, fp32 psum) + one
PSUM->SBUF multiply on a rotating engine path (fused DVE / Act-copy + 2x DVE
mul / Act-copy + GPSIMD mul) to balance engine busy time. Records (rows 96/97
of every ring slot) are DMA'd out; the host converts them to logZ and
subtracts the gold path score.
"""
import os
import sys

import numpy as np

for _p in ("/opt/trn_rl_repo", "/root/.axon_site/_ro/trn_rl_repo"):
    if os.path.isdir(_p) and _p not in sys.path:
        sys.path.insert(0, _p)

import ml_dtypes

import concourse.bacc as bacc
import concourse.tile as tile
from concourse import mybir
from concourse import bass_utils

BF16NP = ml_dtypes.bfloat16

B, S, T = 512, 1024, 48
START, STOP, PAD = 45, 46, 47
NCORE = 8
C = 8                    # chains (time blocks) per core
NPAIR = 4
NST = 24                 # steps per chain
NSLOT = NST + 1          # emis slots per chain (init + 31 em steps)
F = 256                  # sequences per partition-group (one chain-half)
FP = 512                 # pair free width (two chain-halves)
P = 98                   # partitions: 48 tags x 2 groups + 2 stop rows
F32 = mybir.dt.float32
BF16 = mybir.dt.bfloat16

# multiply-path schedule per (pair, round): D = fused DVE (psum x em),
# A = Act copy + 2x-mode DVE mul, P = Act copy + GPSIMD mul.
PAT = "DAPDADAPDAPD"

# emis chunking: chunk 0 = slots [0,2) (small, cuts start stagger), chunks
# 1..5 = 6 slots each (last one 5), triple-buffered: buffer b = k %% 3 at cols
# [6b, 6b+6) slots; chunk 0 nests inside buffer 0.
def _col_of_slot(s):
    if s < 2:
        return s * FP
    k = (s - 2) // 6 + 1
    off = (s - 2) % 6
    return ((k % 3) * 6 + off) * FP


def _chunk_slots(k):
    return (0, 2) if k == 0 else (2 + 6 * (k - 1), min(2 + 6 * k, NSLOT))

_CACHE = {}


def _build_program():
    nc = bacc.Bacc(
        "TRN2",
        target_bir_lowering=False,
        debug=False,
        enable_asserts=False,
        num_devices=NCORE,
    )
    emis_d = nc.dram_tensor(
        "emis", [NPAIR, P, NSLOT * FP], BF16, kind="ExternalInput"
    ).ap()
    w_d = nc.dram_tensor("wts", [96, P], BF16, kind="ExternalInput").ap()
    rec_d = nc.dram_tensor("recs", [NPAIR, 2, NST * FP], BF16, kind="ExternalOutput").ap()

    with tile.TileContext(nc) as tc:
        with tc.tile_pool(name="main", bufs=1) as pool, tc.tile_pool(
            name="ps", bufs=1, space="PSUM"
        ) as pp:
            wt = pool.tile([96, P], BF16)
            nc.sync.dma_start(out=wt[:, :], in_=w_d[:, :])
            rings = [
                pool.tile([P, NST * FP], BF16, tag=f"ring{p}", name=f"ring{p}")
                for p in range(NPAIR)
            ]
            ems = [
                pool.tile([P, 18 * FP], BF16, tag=f"em{p}", name=f"em{p}")
                for p in range(NPAIR)
            ]
            scr = [
                pool.tile([P, 2 * FP], BF16, tag=f"scr{p}", name=f"scr{p}")
                for p in range(NPAIR)
            ]

            def issue_chunk(p, k):
                s0, s1 = _chunk_slots(k)
                c0 = _col_of_slot(s0)
                nc.sync.dma_start(
                    out=ems[p][:, c0 : c0 + (s1 - s0) * FP],
                    in_=emis_d[p, :, s0 * FP : s1 * FP],
                )

            for k in (0, 1, 2):
                for p in range(NPAIR):
                    issue_chunk(p, k)

            # chunk k overwrites the buffer holding chunk k-2, so it may only
            # be ISSUED after the program-order point where chunk k-2's last
            # reader was emitted (the tile framework resolves reads against
            # the latest prior writer).
            chunk_issue_round = {2: 3, 7: 4}
            for i in range(NST):
                for p in range(NPAIR):
                    if i in chunk_issue_round:
                        issue_chunk(p, chunk_issue_round[i])
                    ps = pp.tile([P, FP], F32, tag=f"mm{p}")
                    if i == 0:
                        mm_src = ems[p][0:96, 0:FP]
                    else:
                        mm_src = rings[p][0:96, (i - 1) * FP : i * FP]
                    nc.tensor.matmul(ps[:, :], wt[:, :], mm_src, start=True, stop=True)
                    dst = rings[p][:, i * FP : (i + 1) * FP]
                    cs = _col_of_slot(i + 1)
                    emsl = ems[p][:, cs : cs + FP]
                    mv = PAT[(i + 3 * p) % len(PAT)]
                    if mv == "D":
                        nc.vector.tensor_mul(dst, ps[:, :], emsl)
                    else:
                        sc = scr[p][:, (i % 2) * FP : (i % 2) * FP + FP]
                        nc.scalar.copy(sc, ps[:, :])
                        if mv == "P":
                            nc.gpsimd.tensor_mul(dst[:, 0:F], sc[:, 0:F], emsl[:, 0:F])
                            nc.vector.tensor_mul(dst[:, F:FP], sc[:, F:FP], emsl[:, F:FP])
                        else:
                            nc.vector.tensor_mul(dst, sc, emsl)
                    if i == NST // 2:
                        nc.sync.dma_start(
                            out=rec_d[p, :, 0 : (NST // 2 + 1) * FP],
                            in_=rings[p][96:98, 0 : (NST // 2 + 1) * FP],
                        )
                    elif i == NST - 1:
                        nc.sync.dma_start(
                            out=rec_d[p, :, (NST // 2 + 1) * FP : NST * FP],
                            in_=rings[p][96:98, (NST // 2 + 1) * FP : NST * FP],
                        )

    nc.compile()
    return nc


def _blocks_for_core(k):
    """(a, t0, t1) per chain: stream = em steps [a, a+NST); owned = (t0, t1]."""
    owned = [23, 15, 15, 15, 15, 15, 15, 15] if k == 0 else [16] * 8
    out = []
    t1 = 128 * k
    for o in owned:
        t1 += o
        out.append((t1 - (NST - 1), t1 - o, t1))
    return out


def kernel(feats, masks, tags, transitions):
    feats = np.asarray(feats, dtype=np.float32)
    masks = np.asarray(masks, dtype=np.float32)
    tags = np.asarray(tags)
    trans = np.asarray(transitions, dtype=np.float32)

    if "nc" not in _CACHE:
        _CACHE["nc"] = _build_program()
    nc = _CACHE["nc"]

    lengths = masks.sum(1).astype(np.int64)

    # host prescale: em = exp(feats - LSE_tags(feats)); cumulative C added back
    mx = feats.max(2)
    Kp = np.log(np.exp(feats - mx[:, :, None]).sum(2)) + mx
    Cc = np.zeros((B, S + 1), np.float64)
    Cc[:, 1:] = np.cumsum(Kp.astype(np.float64), 1)
    em = np.exp(feats - Kp[:, :, None].astype(np.float32))

    # packed per-step emission pages [S+1, 98, 256] (page S is a dummy for the
    # one-past-the-end step of the final chain)
    base = np.ones((S + 1, P, F), np.float32)
    base[:S, 0:48] = em[0:F].transpose(1, 2, 0)
    base[:S, 48:96] = em[F:B].transpose(1, 2, 0)

    Mexp = np.exp(trans.astype(np.float64))
    w = np.exp(trans[STOP].astype(np.float64))
    W2 = np.zeros((96, P), np.float64)
    W2[0:48, 0:48] = Mexp.T
    W2[48:96, 48:96] = Mexp.T
    W2[0:48, 96] = w
    W2[48:96, 97] = w
    wts = W2.astype(BF16NP)

    init_uni = np.zeros((P, F), np.float32)
    init_uni[0:96] = 1.0
    init_exact = np.zeros((P, F), np.float32)
    init_exact[START] = 1.0
    init_exact[48 + START] = 1.0

    in_maps = []
    for k in range(NCORE):
        blocks = _blocks_for_core(k)
        # emis[p] slot layout: [NSLOT slots x 512] where cols h*256..h*256+255
        # of slot s belong to chain 2p+h (slot 0 = init, slots 1.. = em pages)
        emv = np.empty((NPAIR, P, NSLOT, 2, F), dtype=BF16NP)
        for c, (a, t0, t1) in enumerate(blocks):
            p, h = divmod(c, 2)
            ini = init_exact if (k == 0 and a == 0) else init_uni
            emv[p, :, 0, h] = ini.astype(BF16NP)
            sl = base[a : a + NST].transpose(1, 0, 2)  # [98, 31, 256]
            emv[p, :, 1:, h] = sl.astype(BF16NP)
        in_maps.append({"emis": emv.reshape(NPAIR, P, NSLOT * FP), "wts": wts})

    _CACHE["in_maps"] = in_maps
    res = bass_utils.run_bass_kernel_spmd(nc, in_maps, core_ids=list(range(NCORE)))
    results = res.results

    # host: stitch per-chain scale offsets, read logZ at L, subtract gold
    chains = []
    for k in range(NCORE):
        rec = (
            np.asarray(results[k]["recs"])
            .astype(np.float64)
            .reshape(NPAIR, 2, NST, 2, F)
        )
        for c, (a, t0, t1) in enumerate(_blocks_for_core(k)):
            p, h = divmod(c, 2)
            chains.append((a, t0, t1, rec[p, :, :, h, :]))
    chains.sort(key=lambda x: x[2])

    grp = np.arange(B) // F
    lane = np.arange(B) % F

    def logr(rc, t, a):
        return np.log(np.maximum(rc[grp, t - a, lane], 1e-300))

    g_off = np.zeros(B)
    logZ = np.full(B, np.nan)
    prev = None
    for (a, t0, t1, rc) in chains:
        if prev is not None:
            pa, _, _, prc = prev
            lt_prev = logr(prc, t0, pa) + Cc[:, t0] - Cc[:, pa] + g_off
            g_off = lt_prev - (logr(rc, t0, a) + Cc[:, t0] - Cc[:, a])
        sel = (lengths > t0) & (lengths <= t1)
        if sel.any():
            Ls = lengths[sel]
            logZ[sel] = (
                np.log(np.maximum(rc[grp[sel], Ls - a, lane[sel]], 1e-300))
                + Cc[sel, Ls]
                - Cc[sel, a]
                + g_off[sel]
            )
        prev = (a, t0, t1, rc)

    bi = np.arange(B)
    em_g = feats[bi[:, None], np.arange(S)[None, :], tags].astype(np.float64)
    tags_ext = np.concatenate([np.full((B, 1), START, tags.dtype), tags], 1)
    trsc = trans.astype(np.float64)[tags_ext[:, 1:], tags_ext[:, :-1]]
    gold = ((em_g + trsc) * masks.astype(np.float64)).sum(1) + trans[
        STOP, tags_ext[bi, lengths]
    ].astype(np.float64)
    return (logZ - gold).astype(np.float32)


# revision 13
# speedup vs baseline: 1.2490x; 1.0034x over previous
"""CRF NLL loss kernel for 8 Trainium2 NeuronCores — time-sharded forward algorithm.

Math: exp-domain forward recurrence alpha_{s+1} = diag(em_s) M alpha_s with
M = exp(transitions), em prescaled per step by its LSE so fp32/bf16 never
over/underflows. logZ(b) = log(w . alpha_{L_b}) with w = exp(trans[STOP]).

Sharding: TIME-sharded (not batch): each core owns a 128-step range of ALL 512
sequences, split into C=6 chains of 31 steps. A chain's stream starts ~9-11
steps before its owned block; the CRF transfer recurrence contracts initial-
condition error by ~0.2x/step (measured), so after the warmup prefix the state
direction is exact to ~1e-7 and only an unknown per-sequence log-scale remains.
The host stitches those scales chain-to-chain through overlap records. Chain 0
of core 0 starts from the exact one-hot START state (no warmup).

Layout: two 256-sequence groups packed on partitions 0-47 / 48-95 plus two
stop-dot rows (96/97) via a block-diagonal [96,98] weight augmented with the
STOP row. Chains are PAIRED: a pair shares one PSUM bank [98,512] (each
chain-half writes 256 columns) so one [98,512] elementwise multiply retires
two chains' steps, halving per-instruction PSUM access penalties and
instruction count. Per pair-step: 2 PE matmuls (bf16, fp32 psum) + one
PSUM->SBUF multiply on a rotating engine path (fused DVE / Act-copy + 2x DVE
mul / Act-copy + GPSIMD mul) to balance engine busy time. Records (rows 96/97
of every ring slot) are DMA'd out; the host converts them to logZ and
subtracts the gold path score.
"""
import os
import sys

import numpy as np

for _p in ("/opt/trn_rl_repo", "/root/.axon_site/_ro/trn_rl_repo"):
    if os.path.isdir(_p) and _p not in sys.path:
        sys.path.insert(0, _p)

import ml_dtypes

import concourse.bacc as bacc
import concourse.tile as tile
from concourse import mybir
from concourse import bass_utils

BF16NP = ml_dtypes.bfloat16

B, S, T = 512, 1024, 48
START, STOP, PAD = 45, 46, 47
NCORE = 8
C = 8                    # chains (time blocks) per core
NPAIR = 4
NST = 23                 # steps per chain
NSLOT = NST + 1          # emis slots per chain (init + 31 em steps)
F = 256                  # sequences per partition-group (one chain-half)
FP = 512                 # pair free width (two chain-halves)
P = 98                   # partitions: 48 tags x 2 groups + 2 stop rows
F32 = mybir.dt.float32
BF16 = mybir.dt.bfloat16

# multiply-path schedule per (pair, round): D = fused DVE (psum x em),
# A = Act copy + 2x-mode DVE mul, P = Act copy + GPSIMD mul.
PAT = "DPAPDPADPAPD"

# emis chunking: chunk 0 = slots [0,2) (small, cuts start stagger), chunks
# 1..5 = 6 slots each (last one 5), triple-buffered: buffer b = k %% 3 at cols
# [6b, 6b+6) slots; chunk 0 nests inside buffer 0.
def _col_of_slot(s):
    if s < 2:
        return s * FP
    k = (s - 2) // 6 + 1
    off = (s - 2) % 6
    return ((k % 3) * 6 + off) * FP


def _chunk_slots(k):
    return (0, 2) if k == 0 else (2 + 6 * (k - 1), min(2 + 6 * k, NSLOT))

_CACHE = {}


def _build_program():
    nc = bacc.Bacc(
        "TRN2",
        target_bir_lowering=False,
        debug=False,
        enable_asserts=False,
        num_devices=NCORE,
    )
    emis_d = nc.dram_tensor(
        "emis", [NPAIR, P, NSLOT * FP], BF16, kind="ExternalInput"
    ).ap()
    w_d = nc.dram_tensor("wts", [96, P], BF16, kind="ExternalInput").ap()
    rec_d = nc.dram_tensor("recs", [NPAIR, 2, NST * FP], BF16, kind="ExternalOutput").ap()

    with tile.TileContext(nc) as tc:
        with tc.tile_pool(name="main", bufs=1) as pool, tc.tile_pool(
            name="ps", bufs=1, space="PSUM"
        ) as pp:
            wt = pool.tile([96, P], BF16)
            nc.sync.dma_start(out=wt[:, :], in_=w_d[:, :])
            rings = [
                pool.tile([P, NST * FP], BF16, tag=f"ring{p}", name=f"ring{p}")
                for p in range(NPAIR)
            ]
            ems = [
                pool.tile([P, 18 * FP], BF16, tag=f"em{p}", name=f"em{p}")
                for p in range(NPAIR)
            ]
            scr = [
                pool.tile([P, 2 * FP], BF16, tag=f"scr{p}", name=f"scr{p}")
                for p in range(NPAIR)
            ]

            def issue_chunk(p, k):
                s0, s1 = _chunk_slots(k)
                c0 = _col_of_slot(s0)
                nc.sync.dma_start(
                    out=ems[p][:, c0 : c0 + (s1 - s0) * FP],
                    in_=emis_d[p, :, s0 * FP : s1 * FP],
                )

            for k in (0, 1, 2):
                for p in range(NPAIR):
                    issue_chunk(p, k)

            # chunk k overwrites the buffer holding chunk k-2, so it may only
            # be ISSUED after the program-order point where chunk k-2's last
            # reader was emitted (the tile framework resolves reads against
            # the latest prior writer).
            chunk_issue_round = {2: 3, 7: 4}
            for i in range(NST):
                for p in range(NPAIR):
                    if i in chunk_issue_round:
                        issue_chunk(p, chunk_issue_round[i])
                    ps = pp.tile([P, FP], F32, tag=f"mm{p}")
                    if i == 0:
                        mm_src = ems[p][0:96, 0:FP]
                    else:
                        mm_src = rings[p][0:96, (i - 1) * FP : i * FP]
                    nc.tensor.matmul(ps[:, :], wt[:, :], mm_src, start=True, stop=True)
                    dst = rings[p][:, i * FP : (i + 1) * FP]
                    cs = _col_of_slot(i + 1)
                    emsl = ems[p][:, cs : cs + FP]
                    mv = PAT[(i + 3 * p) % len(PAT)]
                    if mv == "D":
                        nc.vector.tensor_mul(dst, ps[:, :], emsl)
                    else:
                        sc = scr[p][:, (i % 2) * FP : (i % 2) * FP + FP]
                        nc.scalar.copy(sc, ps[:, :])
                        if mv == "P":
                            nc.gpsimd.tensor_mul(dst[:, 0:F], sc[:, 0:F], emsl[:, 0:F])
                            nc.vector.tensor_mul(dst[:, F:FP], sc[:, F:FP], emsl[:, F:FP])
                        else:
                            nc.vector.tensor_mul(dst, sc, emsl)
                    if i == NST // 2:
                        nc.sync.dma_start(
                            out=rec_d[p, :, 0 : (NST // 2 + 1) * FP],
                            in_=rings[p][96:98, 0 : (NST // 2 + 1) * FP],
                        )
                    elif i == NST - 1:
                        nc.sync.dma_start(
                            out=rec_d[p, :, (NST // 2 + 1) * FP : NST * FP],
                            in_=rings[p][96:98, (NST // 2 + 1) * FP : NST * FP],
                        )

    nc.compile()
    return nc


def _blocks_for_core(k):
    """(a, t0, t1) per chain: stream = em steps [a, a+NST); owned = (t0, t1]."""
    owned = [22, 16, 15, 15, 15, 15, 15, 15] if k == 0 else [16] * 8
    out = []
    t1 = 128 * k
    for o in owned:
        t1 += o
        out.append((t1 - (NST - 1), t1 - o, t1))
    return out


def kernel(feats, masks, tags, transitions):
    feats = np.asarray(feats, dtype=np.float32)
    masks = np.asarray(masks, dtype=np.float32)
    tags = np.asarray(tags)
    trans = np.asarray(transitions, dtype=np.float32)

    if "nc" not in _CACHE:
        _CACHE["nc"] = _build_program()
    nc = _CACHE["nc"]

    lengths = masks.sum(1).astype(np.int64)

    # host prescale: em = exp(feats - LSE_tags(feats)); cumulative C added back
    mx = feats.max(2)
    Kp = np.log(np.exp(feats - mx[:, :, None]).sum(2)) + mx
    Cc = np.zeros((B, S + 1), np.float64)
    Cc[:, 1:] = np.cumsum(Kp.astype(np.float64), 1)
    em = np.exp(feats - Kp[:, :, None].astype(np.float32))

    # packed per-step emission pages [S+1, 98, 256] (page S is a dummy for the
    # one-past-the-end step of the final chain)
    base = np.ones((S + 1, P, F), np.float32)
    base[:S, 0:48] = em[0:F].transpose(1, 2, 0)
    base[:S, 48:96] = em[F:B].transpose(1, 2, 0)

    Mexp = np.exp(trans.astype(np.float64))
    w = np.exp(trans[STOP].astype(np.float64))
    W2 = np.zeros((96, P), np.float64)
    W2[0:48, 0:48] = Mexp.T
    W2[48:96, 48:96] = Mexp.T
    W2[0:48, 96] = w
    W2[48:96, 97] = w
    wts = W2.astype(BF16NP)

    init_uni = np.zeros((P, F), np.float32)
    init_uni[0:96] = 1.0
    init_exact = np.zeros((P, F), np.float32)
    init_exact[START] = 1.0
    init_exact[48 + START] = 1.0

    in_maps = []
    for k in range(NCORE):
        blocks = _blocks_for_core(k)
        # emis[p] slot layout: [NSLOT slots x 512] where cols h*256..h*256+255
        # of slot s belong to chain 2p+h (slot 0 = init, slots 1.. = em pages)
        emv = np.empty((NPAIR, P, NSLOT, 2, F), dtype=BF16NP)
        for c, (a, t0, t1) in enumerate(blocks):
            p, h = divmod(c, 2)
            ini = init_exact if (k == 0 and a == 0) else init_uni
            emv[p, :, 0, h] = ini.astype(BF16NP)
            sl = base[a : a + NST].transpose(1, 0, 2)  # [98, 31, 256]
            emv[p, :, 1:, h] = sl.astype(BF16NP)
        in_maps.append({"emis": emv.reshape(NPAIR, P, NSLOT * FP), "wts": wts})

    _CACHE["in_maps"] = in_maps
    res = bass_utils.run_bass_kernel_spmd(nc, in_maps, core_ids=list(range(NCORE)))
    results = res.results

    # host: stitch per-chain scale offsets, read logZ at L, subtract gold
    chains = []
    for k in range(NCORE):
        rec = (
            np.asarray(results[k]["recs"])
            .astype(np.float64)
            .reshape(NPAIR, 2, NST, 2, F)
        )
        for c, (a, t0, t1) in enumerate(_blocks_for_core(k)):
            p, h = divmod(c, 2)
            chains.append((a, t0, t1, rec[p, :, :, h, :]))
    chains.sort(key=lambda x: x[2])

    grp = np.arange(B) // F
    lane = np.arange(B) % F

    def logr(rc, t, a):
        return np.log(np.maximum(rc[grp, t - a, lane], 1e-300))

    g_off = np.zeros(B)
    logZ = np.full(B, np.nan)
    prev = None
    for (a, t0, t1, rc) in chains:
        if prev is not None:
            pa, _, _, prc = prev
            lt_prev = logr(prc, t0, pa) + Cc[:, t0] - Cc[:, pa] + g_off
            g_off = lt_prev - (logr(rc, t0, a) + Cc[:, t0] - Cc[:, a])
        sel = (lengths > t0) & (lengths <= t1)
        if sel.any():
            Ls = lengths[sel]
            logZ[sel] = (
                np.log(np.maximum(rc[grp[sel], Ls - a, lane[sel]], 1e-300))
                + Cc[sel, Ls]
                - Cc[sel, a]
                + g_off[sel]
            )
        prev = (a, t0, t1, rc)

    bi = np.arange(B)
    em_g = feats[bi[:, None], np.arange(S)[None, :], tags].astype(np.float64)
    tags_ext = np.concatenate([np.full((B, 1), START, tags.dtype), tags], 1)
    trsc = trans.astype(np.float64)[tags_ext[:, 1:], tags_ext[:, :-1]]
    gold = ((em_g + trsc) * masks.astype(np.float64)).sum(1) + trans[
        STOP, tags_ext[bi, lengths]
    ].astype(np.float64)
    return (logZ - gold).astype(np.float32)


# revision 15
# speedup vs baseline: 1.2506x; 1.0012x over previous
"""CRF NLL loss kernel for 8 Trainium2 NeuronCores — time-sharded forward algorithm.

Math: exp-domain forward recurrence alpha_{s+1} = diag(em_s) M alpha_s with
M = exp(transitions), em prescaled per step by its LSE so fp32/bf16 never
over/underflows. logZ(b) = log(w . alpha_{L_b}) with w = exp(trans[STOP]).

Sharding: TIME-sharded (not batch): each core owns a 128-step range of ALL 512
sequences, split into C=6 chains of 31 steps. A chain's stream starts ~9-11
steps before its owned block; the CRF transfer recurrence contracts initial-
condition error by ~0.2x/step (measured), so after the warmup prefix the state
direction is exact to ~1e-7 and only an unknown per-sequence log-scale remains.
The host stitches those scales chain-to-chain through overlap records. Chain 0
of core 0 starts from the exact one-hot START state (no warmup).

Layout: two 256-sequence groups packed on partitions 0-47 / 48-95 plus two
stop-dot rows (96/97) via a block-diagonal [96,98] weight augmented with the
STOP row. Chains are PAIRED: a pair shares one PSUM bank [98,512] (each
chain-half writes 256 columns) so one [98,512] elementwise multiply retires
two chains' steps, halving per-instruction PSUM access penalties and
instruction count. Per pair-step: 2 PE matmuls (bf16, fp32 psum) + one
PSUM->SBUF multiply on a rotating engine path (fused DVE / Act-copy + 2x DVE
mul / Act-copy + GPSIMD mul) to balance engine busy time. Records (rows 96/97
of every ring slot) are DMA'd out; the host converts them to logZ and
subtracts the gold path score.
"""
import os
import sys

import numpy as np

for _p in ("/opt/trn_rl_repo", "/root/.axon_site/_ro/trn_rl_repo"):
    if os.path.isdir(_p) and _p not in sys.path:
        sys.path.insert(0, _p)

import ml_dtypes

import concourse.bacc as bacc
import concourse.tile as tile
from concourse import mybir
from concourse import bass_utils

BF16NP = ml_dtypes.bfloat16

B, S, T = 512, 1024, 48
START, STOP, PAD = 45, 46, 47
NCORE = 8
C = 8                    # chains (time blocks) per core
NPAIR = 4
NST = 23                 # steps per chain
NSLOT = NST + 1          # emis slots per chain (init + 31 em steps)
F = 256                  # sequences per partition-group (one chain-half)
FP = 512                 # pair free width (two chain-halves)
P = 98                   # partitions: 48 tags x 2 groups + 2 stop rows
F32 = mybir.dt.float32
BF16 = mybir.dt.bfloat16

# multiply-path schedule per (pair, round): D = fused DVE (psum x em),
# A = Act copy + 2x-mode DVE mul, P = Act copy + GPSIMD mul.
PAT = "DPAPDPADPAPD"

# emis chunking: chunk 0 = slots [0,2) (small, cuts start stagger), chunks
# 1..5 = 6 slots each (last one 5), triple-buffered: buffer b = k %% 3 at cols
# [6b, 6b+6) slots; chunk 0 nests inside buffer 0.
def _col_of_slot(s):
    if s < 2:
        return s * FP
    k = (s - 2) // 6 + 1
    off = (s - 2) % 6
    return ((k % 3) * 6 + off) * FP


def _chunk_slots(k):
    return (0, 2) if k == 0 else (2 + 6 * (k - 1), min(2 + 6 * k, NSLOT))

_CACHE = {}


def _build_program():
    nc = bacc.Bacc(
        "TRN2",
        target_bir_lowering=False,
        debug=False,
        enable_asserts=False,
        num_devices=NCORE,
    )
    emis_d = nc.dram_tensor(
        "emis", [NPAIR, P, NSLOT * FP], BF16, kind="ExternalInput"
    ).ap()
    w_d = nc.dram_tensor("wts", [96, P], BF16, kind="ExternalInput").ap()
    rec_d = nc.dram_tensor("recs", [2, NPAIR, NST * FP], BF16, kind="ExternalOutput").ap()

    with tile.TileContext(nc) as tc:
        with tc.tile_pool(name="main", bufs=1) as pool, tc.tile_pool(
            name="ps", bufs=1, space="PSUM"
        ) as pp:
            wt = pool.tile([96, P], BF16)
            nc.sync.dma_start(out=wt[:, :], in_=w_d[:, :])
            rings = pool.tile([P, NPAIR, NST * FP], BF16, tag="rings", name="rings")
            ems = pool.tile([P, NPAIR, 18 * FP], BF16, tag="ems", name="ems")
            scr = [
                pool.tile([P, 2 * FP], BF16, tag=f"scr{p}", name=f"scr{p}")
                for p in range(NPAIR)
            ]

            def issue_chunk(p, k):
                s0, s1 = _chunk_slots(k)
                c0 = _col_of_slot(s0)
                if p is None:  # all pairs in one DMA
                    nc.sync.dma_start(
                        out=ems[:, :, c0 : c0 + (s1 - s0) * FP],
                        in_=emis_d[:, :, s0 * FP : s1 * FP],
                    )
                else:
                    nc.sync.dma_start(
                        out=ems[:, p, c0 : c0 + (s1 - s0) * FP],
                        in_=emis_d[p, :, s0 * FP : s1 * FP],
                    )

            for p in range(NPAIR):
                issue_chunk(p, 0)
            for p in range(NPAIR):
                issue_chunk(p, 1)
            issue_chunk(None, 2)

            # chunk k overwrites the buffer holding chunk k-2, so it may only
            # be ISSUED after the program-order point where chunk k-2's last
            # reader was emitted (the tile framework resolves reads against
            # the latest prior writer).
            chunk_issue_round = {2: 3, 7: 4}
            for i in range(NST):
                for p in range(NPAIR):
                    if i in chunk_issue_round and p == 0:
                        issue_chunk(None, chunk_issue_round[i])
                    ps = pp.tile([P, FP], F32, tag=f"mm{p}")
                    if i == 0:
                        mm_src = ems[0:96, p, 0:FP]
                    else:
                        mm_src = rings[0:96, p, (i - 1) * FP : i * FP]
                    nc.tensor.matmul(ps[:, :], wt[:, :], mm_src, start=True, stop=True)
                    dst = rings[:, p, i * FP : (i + 1) * FP]
                    cs = _col_of_slot(i + 1)
                    emsl = ems[:, p, cs : cs + FP]
                    mv = PAT[(i + 3 * p) % len(PAT)]
                    if mv == "D":
                        nc.vector.tensor_mul(dst, ps[:, :], emsl)
                    else:
                        sc = scr[p][:, (i % 2) * FP : (i % 2) * FP + FP]
                        nc.scalar.copy(sc, ps[:, :])
                        if mv == "P":
                            nc.gpsimd.tensor_mul(dst[:, 0:F], sc[:, 0:F], emsl[:, 0:F])
                            nc.vector.tensor_mul(dst[:, F:FP], sc[:, F:FP], emsl[:, F:FP])
                        else:
                            nc.vector.tensor_mul(dst, sc, emsl)
                    if i == NST // 2 and p == NPAIR - 1:
                        nc.sync.dma_start(
                            out=rec_d[:, :, 0 : (NST // 2 + 1) * FP],
                            in_=rings[96:98, :, 0 : (NST // 2 + 1) * FP],
                        )
                    elif i == NST - 1 and p == NPAIR - 1:
                        nc.sync.dma_start(
                            out=rec_d[:, :, (NST // 2 + 1) * FP : NST * FP],
                            in_=rings[96:98, :, (NST // 2 + 1) * FP : NST * FP],
                        )

    nc.compile()
    return nc


def _blocks_for_core(k):
    """(a, t0, t1) per chain: stream = em steps [a, a+NST); owned = (t0, t1]."""
    owned = [22, 16, 15, 15, 15, 15, 15, 15] if k == 0 else [16] * 8
    out = []
    t1 = 128 * k
    for o in owned:
        t1 += o
        out.append((t1 - (NST - 1), t1 - o, t1))
    return out


def kernel(feats, masks, tags, transitions):
    feats = np.asarray(feats, dtype=np.float32)
    masks = np.asarray(masks, dtype=np.float32)
    tags = np.asarray(tags)
    trans = np.asarray(transitions, dtype=np.float32)

    if "nc" not in _CACHE:
        _CACHE["nc"] = _build_program()
    nc = _CACHE["nc"]

    lengths = masks.sum(1).astype(np.int64)

    # host prescale: em = exp(feats - LSE_tags(feats)); cumulative C added back
    mx = feats.max(2)
    Kp = np.log(np.exp(feats - mx[:, :, None]).sum(2)) + mx
    Cc = np.zeros((B, S + 1), np.float64)
    Cc[:, 1:] = np.cumsum(Kp.astype(np.float64), 1)
    em = np.exp(feats - Kp[:, :, None].astype(np.float32))

    # packed per-step emission pages [S+1, 98, 256] (page S is a dummy for the
    # one-past-the-end step of the final chain)
    base = np.ones((S + 1, P, F), np.float32)
    base[:S, 0:48] = em[0:F].transpose(1, 2, 0)
    base[:S, 48:96] = em[F:B].transpose(1, 2, 0)

    Mexp = np.exp(trans.astype(np.float64))
    w = np.exp(trans[STOP].astype(np.float64))
    W2 = np.zeros((96, P), np.float64)
    W2[0:48, 0:48] = Mexp.T
    W2[48:96, 48:96] = Mexp.T
    W2[0:48, 96] = w
    W2[48:96, 97] = w
    wts = W2.astype(BF16NP)

    init_uni = np.zeros((P, F), np.float32)
    init_uni[0:96] = 1.0
    init_exact = np.zeros((P, F), np.float32)
    init_exact[START] = 1.0
    init_exact[48 + START] = 1.0

    in_maps = []
    for k in range(NCORE):
        blocks = _blocks_for_core(k)
        # emis[p] slot layout: [NSLOT slots x 512] where cols h*256..h*256+255
        # of slot s belong to chain 2p+h (slot 0 = init, slots 1.. = em pages)
        emv = np.empty((NPAIR, P, NSLOT, 2, F), dtype=BF16NP)
        for c, (a, t0, t1) in enumerate(blocks):
            p, h = divmod(c, 2)
            ini = init_exact if (k == 0 and a == 0) else init_uni
            emv[p, :, 0, h] = ini.astype(BF16NP)
            sl = base[a : a + NST].transpose(1, 0, 2)  # [98, 31, 256]
            emv[p, :, 1:, h] = sl.astype(BF16NP)
        in_maps.append({"emis": emv.reshape(NPAIR, P, NSLOT * FP), "wts": wts})

    _CACHE["in_maps"] = in_maps
    res = bass_utils.run_bass_kernel_spmd(nc, in_maps, core_ids=list(range(NCORE)))
    results = res.results

    # host: stitch per-chain scale offsets, read logZ at L, subtract gold
    chains = []
    for k in range(NCORE):
        rec = (
            np.asarray(results[k]["recs"])
            .astype(np.float64)
            .reshape(2, NPAIR, NST, 2, F)
        )
        for c, (a, t0, t1) in enumerate(_blocks_for_core(k)):
            p, h = divmod(c, 2)
            chains.append((a, t0, t1, rec[:, p, :, h, :]))
    chains.sort(key=lambda x: x[2])

    grp = np.arange(B) // F
    lane = np.arange(B) % F

    def logr(rc, t, a):
        return np.log(np.maximum(rc[grp, t - a, lane], 1e-300))

    g_off = np.zeros(B)
    logZ = np.full(B, np.nan)
    prev = None
    for (a, t0, t1, rc) in chains:
        if prev is not None:
            pa, _, _, prc = prev
            lt_prev = logr(prc, t0, pa) + Cc[:, t0] - Cc[:, pa] + g_off
            g_off = lt_prev - (logr(rc, t0, a) + Cc[:, t0] - Cc[:, a])
        sel = (lengths > t0) & (lengths <= t1)
        if sel.any():
            Ls = lengths[sel]
            logZ[sel] = (
                np.log(np.maximum(rc[grp[sel], Ls - a, lane[sel]], 1e-300))
                + Cc[sel, Ls]
                - Cc[sel, a]
                + g_off[sel]
            )
        prev = (a, t0, t1, rc)

    bi = np.arange(B)
    em_g = feats[bi[:, None], np.arange(S)[None, :], tags].astype(np.float64)
    tags_ext = np.concatenate([np.full((B, 1), START, tags.dtype), tags], 1)
    trsc = trans.astype(np.float64)[tags_ext[:, 1:], tags_ext[:, :-1]]
    gold = ((em_g + trsc) * masks.astype(np.float64)).sum(1) + trans[
        STOP, tags_ext[bi, lengths]
    ].astype(np.float64)
    return (logZ - gold).astype(np.float32)
